# revision 18
# baseline (speedup 1.0000x reference)
"""Trainium2 Bass kernel for CropConv: 3x3 same-padding conv (64->64 ch) on
[16, 64, 128, 128] fp32 input, with a static crop mask zeroing output rows/cols
[44:84).

Strategy (data-parallel over batch, 8 cores x 2 images each):
  - Host marshals x into a zero-padded row-major layout with row stride 129
    (131 padded rows), so every conv tap (kh, kw) of an output row-chunk is one
    contiguous rhs slice.  Image 0 in SBUF partitions 0-63 (partition =
    in-channel), image 1 in partitions 64-127.
  - The conv is 9 PSUM-accumulated TensorE matmuls per output chunk; four
    64x64 matmuls run concurrently in the four quadrants of the PE array
    (row-half = image, col-half = chunk pairing (c, c+22)).
  - PSUM -> SBUF stage eviction as a single 128-partition DVE copy per PSUM
    tile with fp32 -> bf16 conversion; crop-mask memsets on GpSimd; bf16
    output stores stream on the scalar HWDGE ring interleaved with compute
    (12-row pieces); input loads stream on the sync ring in 10 prioritized
    segments so compute starts early.  Host upcasts the bf16 output to fp32.
"""

import numpy as np

# ---- problem constants (hardcoded; kernel.py must be self-contained) ----
B, C, H, W = 16, 64, 128, 128
OC, KS = 64, 3
N_CORES = 8
IMGS = B // N_CORES  # 2 images per core

WP = W + 1            # padded row stride: 129
HP = H + 3            # padded rows in the x buffer: 131
XLEN = HP * WP        # 16899 elems per partition

RPC = 3               # output rows per chunk
NCH = (H + RPC - 1) // RPC          # 43 chunks per image (last has 2 rows)
NPAIR = 21            # chunk pairs (c, c+22); chunk 21 is the leftover
CHN = RPC * WP        # matmul free dim per full chunk: 387
BAND = 66             # stage rows per band (band0 = y rows 0-65 in
                      # partitions 0-63, band1 = y rows 66-127 in 64-127)
STLEN = IMGS * BAND * W  # stage free size per partition: 16896

CROP0, CROP1 = 44, 84  # masked rows/cols [44, 84)

_CACHE = {}


def _build_module():
    import concourse.tile as tile
    from concourse import bacc, mybir

    f32 = mybir.dt.float32
    bf16 = mybir.dt.bfloat16

    nc = bacc.Bacc("TRN2", target_bir_lowering=False, debug=False,
                   num_devices=N_CORES)

    x_ap = nc.dram_tensor("xin", [IMGS, C, XLEN], bf16,
                          kind="ExternalInput").ap()
    w_ap = nc.dram_tensor("wt", [2 * C, KS * KS * OC], bf16,
                          kind="ExternalInput").ap()
    y_ap = nc.dram_tensor("yout", [IMGS, OC, H, W], bf16,
                          kind="ExternalOutput").ap()

    x_bc = x_ap.rearrange("b c l -> (b c) l")  # [128, XLEN]

    with tile.TileContext(nc) as tc:
        with tc.tile_pool(name="big", bufs=1) as big, \
             tc.tile_pool(name="psum", bufs=8, space="PSUM") as pp:

            x_sb = big.tile([128, XLEN], bf16, tag="xbuf")
            stage = big.tile([128, STLEN], bf16, tag="stage")
            w_sb = big.tile([128, KS * KS * OC], bf16, tag="wbuf")
            scr = big.tile([128, 512], bf16, tag="scratch")

            # [p, img, band-row, col]
            st4 = stage.rearrange("p (i h w) -> p i h w", i=IMGS, w=W)

            # weights (pre-duplicated on host into both partition halves) on
            # the scalar HWDGE ring, concurrent with the first x loads below
            nc.scalar.dma_start(out=w_sb, in_=w_ap)

            # x loads: contiguous padded-row segments, ordered to match the
            # pair schedule below (pairs 0,1,2 then 20/leftover then 3..)
            segs = [(0, 6), (65, 71), (6, 18), (71, 83), (49, 65),
                    (115, 131), (18, 33), (83, 99), (33, 49), (99, 115)]
            for (a, b_) in segs:
                nc.sync.dma_start(out=x_sb[:, a * WP:b_ * WP],
                                  in_=x_bc[:, a * WP:b_ * WP])

            # PE warm-up: a gap-free stream of small dummy matmuls on scratch
            # while the first x segments stream in, so HAM un-throttles
            # (K=8/8) by the time the real matmuls start, and the real ones
            # queue at most ~100 ns behind the tail of the warm-up stream.
            nc.gpsimd.memset(scr[:, 0:128], 0.0)
            pw = pp.tile([128, 512], f32, tag="ps")
            for _ in range(30):
                nc.tensor.matmul(pw[:, 0:128], scr[:, 0:128], scr[:, 0:128],
                                 start=True, stop=True, skip_group_check=True)

            def lhsT(half, t):
                return w_sb[half * 64:(half + 1) * 64, t * OC:(t + 1) * OC]

            def rhs(half, c, kh, kw, n):
                off = (RPC * c + kh) * WP + kw
                return x_sb[half * 64:(half + 1) * 64, off:off + n]

            TAPS = [(kh, kw) for kh in range(KS) for kw in range(KS)]

            def store_piece(band, r0, nr, eng):
                # one store per (band, row range): 64 partitions (= oc),
                # free dims (img, rows, cols); dst y rows offset by 66*band
                src = st4[band * 64:band * 64 + 64, :, r0:r0 + nr, :]
                yr0 = BAND * band + r0
                dst = y_ap[:, :, yr0:yr0 + nr, :].rearrange(
                    "b o h w -> o b h w")
                eng.dma_start(out=dst, in_=src)

            def mask_memset(band, r0, r1):
                for i in range(IMGS):
                    nc.gpsimd.memset(
                        st4[band * 64:band * 64 + 64, i, r0:r1,
                            CROP0:CROP1], 0.0)

            def do_pair(c, split_evict=False):
                c2 = c + 22
                n2 = 2 * WP if c2 == NCH - 1 else CHN  # 258 for chunk 42
                pa = pp.tile([128, 512], f32, tag="ps")
                pb = pp.tile([128, 512], f32, tag="ps")
                for t, (kh, kw) in enumerate(TAPS):
                    st, sp = (t == 0), (t == len(TAPS) - 1)
                    # img0 chunk c -> A[0:64];  img0 chunk c+22 -> A[64:128]
                    nc.tensor.matmul(pa[0:64, 0:CHN], lhsT(0, t),
                                     rhs(0, c, kh, kw, CHN), start=st, stop=sp,
                                     skip_group_check=True)
                    nc.tensor.matmul(pa[64:128, 0:n2], lhsT(0, t),
                                     rhs(0, c2, kh, kw, n2), start=st, stop=sp,
                                     skip_group_check=True)
                    # img1 chunk c -> B[0:64];  img1 chunk c+22 -> B[64:128]
                    nc.tensor.matmul(pb[0:64, 0:CHN], lhsT(1, t),
                                     rhs(1, c, kh, kw, CHN), start=st, stop=sp,
                                     skip_group_check=True)
                    nc.tensor.matmul(pb[64:128, 0:n2], lhsT(1, t),
                                     rhs(1, c2, kh, kw, n2), start=st, stop=sp,
                                     skip_group_check=True)

                # evict PSUM -> stage: band0 rows 3c..3c+2 (partitions 0-63)
                # and band1 rows 3c..3c+2 (partitions 64-127) share the same
                # free offset -> one 128-partition DVE copy per PSUM tile
                pa3 = pa[:, 0:CHN].rearrange("p (h w) -> p h w", w=WP)
                pb3 = pb[:, 0:CHN].rearrange("p (h w) -> p h w", w=WP)
                if c2 != NCH - 1:
                    nc.vector.tensor_copy(st4[:, 0, 3 * c:3 * c + 3, :],
                                          pa3[:, 0:3, 0:W])
                    if split_evict:
                        # last pair: evict img1 on ScalarE in parallel so the
                        # kernel-tail chain is one cast, not two
                        nc.scalar.copy(st4[:, 1, 3 * c:3 * c + 3, :],
                                       pb3[:, 0:3, 0:W])
                    else:
                        nc.vector.tensor_copy(st4[:, 1, 3 * c:3 * c + 3, :],
                                              pb3[:, 0:3, 0:W])
                else:
                    # chunk 42 has only 2 rows -> split this eviction
                    for i, p3 in ((0, pa3), (1, pb3)):
                        nc.vector.tensor_copy(
                            st4[0:64, i, 3 * c:3 * c + 3, :],
                            p3[0:64, 0:3, 0:W])
                        nc.vector.tensor_copy(
                            st4[64:128, i, 3 * c:3 * c + 2, :],
                            p3[64:128, 0:2, 0:W])

            def do_leftover():
                # chunk 21 (y rows 63-65), both images, via two banks
                pc_ = pp.tile([128, 512], f32, tag="ps")
                pd_ = pp.tile([128, 512], f32, tag="ps")
                for t, (kh, kw) in enumerate(TAPS):
                    st, sp = (t == 0), (t == len(TAPS) - 1)
                    nc.tensor.matmul(pc_[0:64, 0:CHN], lhsT(0, t),
                                     rhs(0, 21, kh, kw, CHN), start=st,
                                     stop=sp, skip_group_check=True)
                    nc.tensor.matmul(pd_[0:64, 0:CHN], lhsT(1, t),
                                     rhs(1, 21, kh, kw, CHN), start=st,
                                     stop=sp, skip_group_check=True)
                pc3 = pc_[:, 0:CHN].rearrange("p (h w) -> p h w", w=WP)
                pd3 = pd_[:, 0:CHN].rearrange("p (h w) -> p h w", w=WP)
                nc.vector.tensor_copy(st4[0:64, 0, 63:66, :],
                                      pc3[0:64, 0:3, 0:W])
                nc.vector.tensor_copy(st4[0:64, 1, 63:66, :],
                                      pd3[0:64, 0:3, 0:W])

            # Pair order [0,1,2, 20, leftover, 3..11, 14..19, 12,13]: the
            # tail rows (60-65 / 126-127) are computed and stored early, and
            # the kernel ends on pairs 12/13 whose 6-row piece (y rows 36-41
            # / 102-107) is unmasked -- the final chain is one eviction cast
            # plus one small store, with no memset in between.  Store piece
            # [r0, r1) of both bands fires as soon as its chunks are evicted;
            # masked y rows [44,84) get GpSimd memsets right before their
            # piece's stores.  band0 stores ride the scalar ring, band1 sync.
            do_pair(0)
            do_pair(1)
            mask_memset(1, 0, 6)            # y rows 66-71
            store_piece(0, 0, 6, nc.scalar)
            store_piece(1, 0, 6, nc.sync)
            do_pair(2)
            do_pair(3)
            mask_memset(1, 6, 12)           # y rows 72-77
            store_piece(0, 6, 6, nc.scalar)
            store_piece(1, 6, 6, nc.sync)
            do_pair(20)
            do_leftover()
            mask_memset(0, 60, 66)          # y rows 60-65
            store_piece(0, 60, 6, nc.scalar)
            store_piece(1, 60, 2, nc.sync)  # y rows 126-127
            # (band, r0, r1) memsets due right before piece [6k, 6k+6)
            piece_memsets = {
                2: [(1, 12, 18)],         # y rows 78-83
            }
            for c in list(range(4, 12)) + list(range(14, 20)):
                do_pair(c)
                if c % 2 == 1 and c <= 11:
                    k = c // 2
                    for (band, r0, r1) in piece_memsets.get(k, []):
                        mask_memset(band, r0, r1)
                    store_piece(0, 6 * k, 6, nc.scalar)
                    store_piece(1, 6 * k, 6, nc.sync)
                elif c == 15:
                    mask_memset(0, 44, 48)
                    store_piece(0, 42, 6, nc.scalar)
                    store_piece(1, 42, 6, nc.sync)
                elif c == 17:
                    mask_memset(0, 48, 54)
                    store_piece(0, 48, 6, nc.scalar)
                    store_piece(1, 48, 6, nc.sync)
                elif c == 18:
                    mask_memset(0, 54, 57)
                    store_piece(0, 54, 3, nc.scalar)
                    store_piece(1, 54, 3, nc.sync)
                elif c == 19:
                    mask_memset(0, 57, 60)
                    store_piece(0, 57, 3, nc.scalar)
                    store_piece(1, 57, 3, nc.sync)
            do_pair(12)
            do_pair(13, split_evict=True)
            # final piece: y rows 36-41 / 102-107, unmasked
            store_piece(0, 36, 6, nc.scalar)
            store_piece(1, 36, 6, nc.sync)

    nc.compile()
    return nc


def _get_module():
    if "nc" not in _CACHE:
        _CACHE["nc"] = _build_module()
    return _CACHE["nc"]


def _make_in_maps(x, weight):
    x = np.asarray(x, dtype=np.float32)
    weight = np.asarray(weight, dtype=np.float32)
    # host marshaling: pad x into the row-major stride-129 layout
    xp = np.zeros((B, C, HP, WP), dtype=np.float32)
    xp[:, :, 1:H + 1, 1:W + 1] = x
    xp = xp.reshape(B, C, XLEN)
    import ml_dtypes
    xp = xp.astype(ml_dtypes.bfloat16)
    # weight [oc, ic, kh, kw] -> [ic, (kh kw), oc], duplicated into both
    # partition halves so a single 128-partition DMA loads it
    wt = np.ascontiguousarray(
        weight.transpose(1, 2, 3, 0).reshape(C, KS * KS * OC)
    ).astype(ml_dtypes.bfloat16)
    wt2 = np.concatenate([wt, wt], axis=0)
    return [
        {"xin": np.ascontiguousarray(xp[k * IMGS:(k + 1) * IMGS]), "wt": wt2}
        for k in range(N_CORES)
    ]


def kernel(x, weight):
    from concourse.bass_utils import run_bass_kernel_spmd

    nc = _get_module()
    in_maps = _make_in_maps(x, weight)
    res = run_bass_kernel_spmd(nc, in_maps, list(range(N_CORES)))
    out = np.concatenate([res.results[k]["yout"] for k in range(N_CORES)],
                         axis=0)
    return out.astype(np.float32)
